# revision 12
# baseline (speedup 1.0000x reference)
"""Trainium2 Bass kernel for nn_MultiHeadAttention_73272142069863.

Reference semantics (softmax over the HEADS axis, dim=-1 of [b,i,j,h]):

    q = (query @ Wq).reshape(B, S, H, DH)        # biases are zero
    k = (key   @ Wk).reshape(B, S, H, DH)
    v = (value @ Wv).reshape(B, S, H, DH)
    scores = einsum("bihd,bjhd->bijh", q, k) / sqrt(DH)
    attn = softmax(scores, axis=-1)              # over h!
    x = einsum("bijh,bjhd->bihd", attn, v).reshape(B, S, D)
    out = x @ Wo

Sharding: core c handles batch b = c // 2 and query-row half ih = c % 2
(I = 512 rows). K/V work duplicated across the pair; no collectives.

Pair-major pipeline (vs the jb-major baseline): for each head-pair g,
project Q/K chunk g then immediately run scores(g, jb) + exp, so the
ACT engine starts ~6us into the kernel instead of ~45us, and the PE
always has independent projection work queued (stays HAM-warm).
jb-halves bound E-tile SBUF residency; pairs 0-3 accumulate x^T in
persistent PSUM during the B phases, pairs 4-7 replay from stored
normalized attn in the tail (PSUM is 8 banks).
"""

import numpy as np
import ml_dtypes

import concourse.bass as bass
import concourse.bacc as bacc
import concourse.tile as tile
from concourse import mybir
from concourse.bass_utils import run_bass_kernel_spmd

B, S, D, H = 4, 1024, 1024, 16
DH = D // H  # 64
SCALE = 1.0 / float(np.sqrt(DH))
I = 512          # query rows per core
NCORES = 8
KC = D // 128    # 8 contraction chunks
JB = S // 128    # 8 j blocks
NPAIR = H // 2   # 8 head pairs

BF16 = mybir.dt.bfloat16
F32 = mybir.dt.float32
EXP = mybir.ActivationFunctionType.Exp


def _build():
    nc = bacc.Bacc(target_bir_lowering=False, trn_type="TRN2")

    q_d = nc.dram_tensor("qT", [D, I], BF16, kind="ExternalInput")
    k_d = nc.dram_tensor("kT", [D, S], BF16, kind="ExternalInput")
    v_d = nc.dram_tensor("vT", [D, S], BF16, kind="ExternalInput")
    # wq/wk packed g-major: [g, kc, 128, 128] -> [8192, 128]; slice for
    # head-pair g is rows g*1024:(g+1)*1024 (8 stacked [128,128] tiles).
    wqp_d = nc.dram_tensor("wqp", [KC * D, 128], BF16, kind="ExternalInput")
    wkp_d = nc.dram_tensor("wkp", [KC * D, 128], BF16, kind="ExternalInput")
    wv_d = nc.dram_tensor("wv", [D, D], BF16, kind="ExternalInput")
    wo_d = nc.dram_tensor("wo", [D, D], BF16, kind="ExternalInput")
    out_d = nc.dram_tensor("out", [I, D], F32, kind="ExternalOutput")
    # HBM spill for normalized attn pairs 4-7, j-blocks 0-3 (SBUF relief)
    eh_d = nc.dram_tensor("eh_spill", [4 * 128, 4096], BF16, kind="Internal")

    with tile.TileContext(nc) as tc:
        with (
            tc.tile_pool(name="persist", bufs=KC) as pp,
            tc.tile_pool(name="ps_rot", bufs=2, space="PSUM") as ps_rot,
            tc.tile_pool(name="ps_xt", bufs=4, space="PSUM") as ps_xt,
            tc.tile_pool(name="pe_lo", bufs=5) as pe_lo,
            tc.tile_pool(name="pe_hi", bufs=5) as pe_hi,
            tc.tile_pool(name="pz", bufs=1) as pz,
            tc.tile_pool(name="pscr", bufs=1) as pscr,
        ):
            # ---- persistent tiles -------------------------------------
            QTs = [pp.tile([128, I], BF16, tag="QTs", name="QTs") for _ in range(KC)]
            KTs = [pp.tile([128, S], BF16, tag="KTs", name="KTs") for _ in range(KC)]
            Vs = [pp.tile([128, D], BF16, tag="Vs", name="Vs") for _ in range(JB)]
            xTs = [pp.tile([128, I], BF16, tag="xTs", name="xTs") for _ in range(NPAIR)]

            E_lo = [None] * JB
            E_hi = [None] * JB

            def v_proj(jc, vT, wv_t):
                js = slice(jc * 128, (jc + 1) * 128)
                ps = ps_rot.tile([128, S], F32, tag="rot", name="rot")
                for kc in range(KC):
                    for nh in range(2):
                        nsl = slice(nh * 512, (nh + 1) * 512)
                        nc.tensor.matmul(
                            ps[:, nsl], vT[kc][:, js], wv_t[kc][:, nsl],
                            start=(kc == 0), stop=(kc == KC - 1),
                        )
                nc.vector.tensor_copy(Vs[jc][:], ps[:])

            def scores_exp(g, jb, e_dst):
                """score matmuls for head pair g, j-block jb + exp."""
                jsl = slice(jb * 128, (jb + 1) * 128)
                sc = ps_rot.tile([128, S], F32, tag="rot", name="rot")
                nc.tensor.matmul(
                    sc[:, 0:512], KTs[g][0:64, jsl], QTs[g][0:64, :],
                    start=True, stop=True, tile_position=(0, 0),
                )
                nc.tensor.matmul(
                    sc[:, 512:1024], KTs[g][64:128, jsl], QTs[g][64:128, :],
                    start=True, stop=True, tile_position=(64, 0),
                )
                nc.scalar.activation(e_dst, sc[:], EXP, scale=SCALE)

            def alloc_e(jb):
                E_lo[jb] = pe_lo.tile([128, 4096], BF16, tag="Elo", name="Elo")
                E_hi[jb] = pe_hi.tile([128, 4096], BF16, tag="Ehi", name="Ehi")

            def a_phase1(vT, wv_t, q_proj, k_proj):
                """g-major over jb 0-3, with Q/K projections interleaved."""
                for jb in range(4):
                    alloc_e(jb)
                for g in range(NPAIR):
                    q_proj(g)
                    k_proj(g)
                    for jb in range(4):
                        dst = (E_lo[jb] if g < 4 else E_hi[jb])
                        off = (g % 4) * 1024
                        scores_exp(g, jb, dst[:, off : off + 1024])
                    if g % 2 == 1:
                        v_proj(g // 2, vT, wv_t)

            def a2_iter(jb, vT, wv_t):
                """jb-major (projections all done): one j-block's scores."""
                alloc_e(jb)
                for g in range(NPAIR):
                    dst = (E_lo[jb] if g < 4 else E_hi[jb])
                    off = (g % 4) * 1024
                    scores_exp(g, jb, dst[:, off : off + 1024])
                v_proj(jb, vT, wv_t)

            def b_phase(jb, xt_ps, first, last):
                # Z = sum of 16 exp planes, via big strided slice adds
                t2 = pscr.tile([128, 2048], BF16, tag="t2", name="t2")
                nc.vector.tensor_add(t2[:], E_lo[jb][:, 0:2048], E_lo[jb][:, 2048:4096])
                nc.vector.tensor_add(t2[:], t2[:], E_hi[jb][:, 0:2048])
                nc.vector.tensor_add(t2[:], t2[:], E_hi[jb][:, 2048:4096])
                nc.vector.tensor_add(t2[:, 0:1024], t2[:, 0:1024], t2[:, 1024:2048])
                zf = pz.tile([128, 512], F32, tag="zf", name="zf")
                nc.vector.tensor_add(zf[:], t2[:, 0:512], t2[:, 512:1024])
                rf = pz.tile([128, 512], F32, tag="rf", name="rf")
                nc.vector.reciprocal_approx_fast(rf[:], zf[:])
                rb = pz.tile([128, 512], BF16, tag="rb", name="rb", bufs=2)
                nc.vector.tensor_copy(rb[:], rf[:])
                rv8 = rb[:].unsqueeze(1).broadcast_to([128, 8, 512])
                nc.vector.tensor_mul(
                    E_lo[jb][:].rearrange("p (a b) -> p a b", a=8),
                    E_lo[jb][:].rearrange("p (a b) -> p a b", a=8),
                    rv8,
                )
                nc.vector.tensor_mul(
                    E_hi[jb][:].rearrange("p (a b) -> p a b", a=8),
                    E_hi[jb][:].rearrange("p (a b) -> p a b", a=8),
                    rv8,
                )
                if jb < 4:
                    # spill normalized pairs 4-7 attn; read back in the tail
                    nc.sync.dma_start(eh_d[jb * 128 : (jb + 1) * 128, :], E_hi[jb][:])
                for g in range(4):
                    for p in range(2):
                        h = 2 * g + p
                        nc.tensor.matmul(
                            xt_ps[g][p * 64 : (p + 1) * 64, :],
                            Vs[jb][:, h * DH : (h + 1) * DH],
                            E_lo[jb][:, (2 * g + p) * 512 : (2 * g + p + 1) * 512],
                            start=first, stop=last,
                            tile_position=(0, p * 64),
                        )

            with (
                tc.tile_pool(name="st_v", bufs=KC) as pv,
                tc.tile_pool(name="st_wv", bufs=KC) as pwv,
            ):
                vT = [pv.tile([128, S], BF16, tag="vT", name="vT") for _ in range(KC)]
                wv_t = [pwv.tile([128, D], BF16, tag="wv", name="wv") for _ in range(KC)]

                with (
                    tc.tile_pool(name="st_q", bufs=KC) as pq,
                    tc.tile_pool(name="st_k", bufs=KC) as pk,
                    tc.tile_pool(name="st_wq", bufs=2 * KC) as pwq,
                    tc.tile_pool(name="st_wk", bufs=2 * KC) as pwk,
                ):
                    qT = [pq.tile([128, I], BF16, tag="qT", name="qT") for _ in range(KC)]
                    kT = [pk.tile([128, S], BF16, tag="kT", name="kT") for _ in range(KC)]
                    wq_g = [[pwq.tile([128, 128], BF16, tag="wq", name="wq")
                             for _ in range(KC)] for _ in range(NPAIR)]
                    wk_g = [[pwk.tile([128, 128], BF16, tag="wk", name="wk")
                             for _ in range(KC)] for _ in range(NPAIR)]

                    def load_w_g(tiles, dram, g):
                        for c in range(KC):
                            r = g * D + c * 128
                            nc.sync.dma_start(tiles[g][c][:], dram[r : r + 128, :])

                    # loads in consumption order; vT/wv ahead of the late
                    # wq/wk chunks so a stalled weight DMA can't starve V
                    load_w_g(wq_g, wqp_d, 0)
                    for c in range(KC):
                        cs = slice(c * 128, (c + 1) * 128)
                        nc.sync.dma_start(qT[c][:], q_d[cs, :])
                    load_w_g(wk_g, wkp_d, 0)
                    for c in range(KC):
                        cs = slice(c * 128, (c + 1) * 128)
                        nc.sync.dma_start(kT[c][:], k_d[cs, :])
                    load_w_g(wq_g, wqp_d, 1)
                    load_w_g(wk_g, wkp_d, 1)
                    for c in range(KC):
                        cs = slice(c * 128, (c + 1) * 128)
                        nc.sync.dma_start(vT[c][:], v_d[cs, :])
                        nc.sync.dma_start(wv_t[c][:], wv_d[cs, :])
                    for g in range(2, NPAIR):
                        load_w_g(wq_g, wqp_d, g)
                        load_w_g(wk_g, wkp_d, g)

                    def q_proj(g):
                        ps = ps_rot.tile([128, S], F32, tag="rot", name="rot")
                        for kc in range(KC):
                            nc.tensor.matmul(
                                ps[:, 0:I], wq_g[g][kc][:], qT[kc][:],
                                start=(kc == 0), stop=(kc == KC - 1),
                            )
                        nc.vector.tensor_copy(QTs[g][:], ps[:, 0:I])

                    def k_proj(g):
                        ps = ps_rot.tile([128, S], F32, tag="rot", name="rot")
                        for kc in range(KC):
                            for nh in range(2):
                                nsl = slice(nh * 512, (nh + 1) * 512)
                                nc.tensor.matmul(
                                    ps[:, nsl], wk_g[g][kc][:], kT[kc][:, nsl],
                                    start=(kc == 0), stop=(kc == KC - 1),
                                )
                        nc.vector.tensor_copy(KTs[g][:], ps[:])

                    a_phase1(vT, wv_t, q_proj, k_proj)
                # st_q/st_k/st_wq/st_wk freed here

                xt_ps = [ps_xt.tile([128, I], F32, tag="xt", name="xt") for _ in range(4)]
                # interleave A2 (scores/exp jb 4-7 on PE/ACT) with B(0..3)
                # (DVE softmax chains + AV pairs 0-3); a2_iter(jb) issued
                # ahead of b_phase(jb-4) so the in-order PE queue never
                # head-of-line blocks on a DVE chain
                for jb in range(4, JB):
                    a2_iter(jb, vT, wv_t)
                    b_phase(jb - 4, xt_ps, first=(jb == 4), last=False)
            # st_v/st_wv freed here

            for jb in range(4, JB):
                b_phase(jb, xt_ps, first=False, last=(jb == JB - 1))
            for g in range(4):
                nc.scalar.copy(xTs[g][:], xt_ps[g][:])

            # ---- tail: AV pairs 4-7; jb 4-7 from SBUF, jb 0-3 from the
            # HBM spill (read back into a small rotation pool) ----------
            with tc.tile_pool(name="peb", bufs=4) as peb:
                xt_ps2 = [ps_xt.tile([128, I], F32, tag="xt", name="xt") for _ in range(4)]

                def av_hi(jb, src, first, last):
                    for g in range(4, NPAIR):
                        for p in range(2):
                            h = 2 * g + p
                            off = (2 * (g - 4) + p) * 512
                            nc.tensor.matmul(
                                xt_ps2[g - 4][p * 64 : (p + 1) * 64, :],
                                Vs[jb][:, h * DH : (h + 1) * DH],
                                src[:, off : off + 512],
                                start=first, stop=last,
                                tile_position=(0, p * 64),
                            )

                backs = []
                for jb in range(4):
                    eb = peb.tile([128, 4096], BF16, tag="eb", name="eb")
                    nc.sync.dma_start(eb[:], eh_d[jb * 128 : (jb + 1) * 128, :])
                    backs.append(eb)
                for jb in range(4, JB):
                    av_hi(jb, E_hi[jb][:], first=(jb == 4), last=False)
                for jb in range(4):
                    av_hi(jb, backs[jb][:], first=False, last=(jb == 3))
                for g in range(4, NPAIR):
                    nc.scalar.copy(xTs[g][:], xt_ps2[g - 4][:])

            # ---- output projection (wo loaded late) -------------------
            with (
                tc.tile_pool(name="pwo", bufs=KC) as pwo,
                tc.tile_pool(name="pout", bufs=2) as pout,
            ):
                wo_t = [pwo.tile([128, D], BF16, tag="wo", name="wo") for _ in range(KC)]
                for g in range(KC):
                    nc.sync.dma_start(wo_t[g][:], wo_d[g * 128 : (g + 1) * 128, :])
                for ic in range(I // 128):
                    isl = slice(ic * 128, (ic + 1) * 128)
                    ps = ps_rot.tile([128, D], F32, tag="rot", name="rot")
                    for g in range(KC):
                        for nh in range(2):
                            nsl = slice(nh * 512, (nh + 1) * 512)
                            nc.tensor.matmul(
                                ps[:, nsl], xTs[g][:, isl], wo_t[g][:, nsl],
                                start=(g == 0), stop=(g == KC - 1),
                            )
                    of = pout.tile([128, D], F32, tag="outf", name="outf")
                    nc.scalar.copy(of[:], ps[:])
                    nc.sync.dma_start(out_d[isl, :], of[:])
    nc.compile()
    return nc


_NC_CACHE = {}


def _get_nc():
    if "nc" not in _NC_CACHE:
        _NC_CACHE["nc"] = _build()
    return _NC_CACHE["nc"]


def _reference_numpy(query, key, value, mask, Wq, bq, Wk, bk, Wv, bv, Wo, bo):
    """Fallback for masked / biased inputs (reference semantics)."""
    q = (query.reshape(B * S, D) @ Wq + bq).reshape(B, S, H, DH)
    k = (key.reshape(B * S, D) @ Wk + bk).reshape(B, S, H, DH)
    v = (value.reshape(B * S, D) @ Wv + bv).reshape(B, S, H, DH)
    scores = np.einsum("bihd,bjhd->bijh", q, k).astype(np.float32) * SCALE
    scores = np.where(mask[..., None] == 0, -np.inf, scores)
    m = scores.max(axis=-1, keepdims=True)
    e = np.exp(scores - m)
    attn = e / e.sum(axis=-1, keepdims=True)
    x = np.einsum("bijh,bjhd->bihd", attn, v).reshape(B, S, D)
    return (x.reshape(B * S, D) @ Wo + bo).reshape(B, S, D).astype(np.float32)


def _pack_w(w):
    """[D, D] -> [KC*D, 128] g-major: rows g*D+kc*128 hold w[kc-rows, g-cols]."""
    bf = ml_dtypes.bfloat16
    t = w.reshape(KC, 128, NPAIR, 128)  # [kc, 128, g, 128]
    t = np.transpose(t, (2, 0, 1, 3)).reshape(KC * D, 128)
    return np.ascontiguousarray(t.astype(bf))


def kernel(query, key, value, mask, Wq, bq, Wk, bk, Wv, bv, Wo, bo):
    query = np.asarray(query, np.float32)
    key = np.asarray(key, np.float32)
    value = np.asarray(value, np.float32)
    Wq, Wk, Wv, Wo = (np.asarray(w, np.float32) for w in (Wq, Wk, Wv, Wo))
    bq, bk, bv, bo = (np.asarray(b, np.float32) for b in (bq, bk, bv, bo))
    mask_np = np.asarray(mask)

    if (not np.all(mask_np != 0)) or bq.any() or bk.any() or bv.any() or bo.any():
        return _reference_numpy(
            query, key, value, mask_np, Wq, bq, Wk, bk, Wv, bv, Wo, bo
        )

    nc = _get_nc()

    bf = ml_dtypes.bfloat16
    qb = query.astype(bf)
    kTb = [np.ascontiguousarray(key[b].T.astype(bf)) for b in range(B)]
    vTb = [np.ascontiguousarray(value[b].T.astype(bf)) for b in range(B)]
    wqp, wkp = _pack_w(Wq), _pack_w(Wk)
    wvb, wob = Wv.astype(bf), Wo.astype(bf)

    in_maps = []
    for c in range(NCORES):
        b, ih = divmod(c, 2)
        in_maps.append({
            "qT": np.ascontiguousarray(qb[b, ih * I : (ih + 1) * I, :].T),
            "kT": kTb[b], "vT": vTb[b],
            "wqp": wqp, "wkp": wkp, "wv": wvb, "wo": wob,
        })

    res = run_bass_kernel_spmd(nc, in_maps, core_ids=list(range(NCORES)))
    global LAST_RESULT
    LAST_RESULT = res
    out = np.empty((B, S, D), np.float32)
    for c in range(NCORES):
        b, ih = divmod(c, 2)
        out[b, ih * I : (ih + 1) * I, :] = res.results[c]["out"]
    return out


# revision 24
# speedup vs baseline: 1.3878x; 1.3878x over previous
"""Trainium2 Bass kernel for nn_MultiHeadAttention_73272142069863.

Reference semantics (softmax over the HEADS axis, dim=-1 of [b,i,j,h]):

    q = (query @ Wq).reshape(B, S, H, DH)        # biases are zero
    k = (key   @ Wk).reshape(B, S, H, DH)
    v = (value @ Wv).reshape(B, S, H, DH)
    scores = einsum("bihd,bjhd->bijh", q, k) / sqrt(DH)
    attn = softmax(scores, axis=-1)              # over h!
    x = einsum("bijh,bjhd->bihd", attn, v).reshape(B, S, D)
    out = x @ Wo

Sharding: core c handles batch b = c // 2 and query-row half ih = c % 2
(I = 512 rows). K/V work duplicated across the pair; no collectives.

Pair-major pipeline (vs the jb-major baseline): for each head-pair g,
project Q/K chunk g then immediately run scores(g, jb) + exp, so the
ACT engine starts ~6us into the kernel instead of ~45us, and the PE
always has independent projection work queued (stays HAM-warm).
jb-halves bound E-tile SBUF residency; pairs 0-3 accumulate x^T in
persistent PSUM during the B phases, pairs 4-7 replay from stored
normalized attn in the tail (PSUM is 8 banks).
"""

import numpy as np
import ml_dtypes

import concourse.bass as bass
import concourse.bacc as bacc
import concourse.tile as tile
from concourse import mybir
from concourse.bass_utils import run_bass_kernel_spmd

B, S, D, H = 4, 1024, 1024, 16
DH = D // H  # 64
SCALE = 1.0 / float(np.sqrt(DH))
I = 512          # query rows per core
NCORES = 8
KC = D // 128    # 8 contraction chunks
JB = S // 128    # 8 j blocks
NPAIR = H // 2   # 8 head pairs

BF16 = mybir.dt.bfloat16
F32 = mybir.dt.float32
EXP = mybir.ActivationFunctionType.Exp


def _build():
    nc = bacc.Bacc(target_bir_lowering=False, trn_type="TRN2")

    q_d = nc.dram_tensor("qT", [D, I], BF16, kind="ExternalInput")
    k_d = nc.dram_tensor("kT", [D, S], BF16, kind="ExternalInput")
    v_d = nc.dram_tensor("vT", [D, S], BF16, kind="ExternalInput")
    # wq/wk packed g-major: [g, kc, 128, 128] -> [8192, 128]; slice for
    # head-pair g is rows g*1024:(g+1)*1024 (8 stacked [128,128] tiles).
    wqp_d = nc.dram_tensor("wqp", [KC * D, 128], BF16, kind="ExternalInput")
    wkp_d = nc.dram_tensor("wkp", [KC * D, 128], BF16, kind="ExternalInput")
    wv_d = nc.dram_tensor("wv", [D, D], BF16, kind="ExternalInput")
    wo_d = nc.dram_tensor("wo", [D, D], BF16, kind="ExternalInput")
    out_d = nc.dram_tensor("out", [I, D], F32, kind="ExternalOutput")
    # HBM spill for normalized attn pairs 4-7, j-blocks 0-3 (SBUF relief)
    eh_d = nc.dram_tensor("eh_spill", [4 * 128, 4096], BF16, kind="Internal")

    with tile.TileContext(nc) as tc:
        with (
            tc.tile_pool(name="persist", bufs=KC) as pp,
            tc.tile_pool(name="ps_rot", bufs=2, space="PSUM") as ps_rot,
            tc.tile_pool(name="ps_xt", bufs=4, space="PSUM") as ps_xt,
            tc.tile_pool(name="pe_lo", bufs=5) as pe_lo,
            tc.tile_pool(name="pe_hi", bufs=5) as pe_hi,
            tc.tile_pool(name="pz", bufs=1) as pz,
            tc.tile_pool(name="pscr", bufs=1) as pscr,
        ):
            # ---- persistent tiles -------------------------------------
            QTs = [pp.tile([128, I], BF16, tag="QTs", name="QTs") for _ in range(KC)]
            KTs = [pp.tile([128, S], BF16, tag="KTs", name="KTs") for _ in range(KC)]
            Vs = [pp.tile([128, D], BF16, tag="Vs", name="Vs") for _ in range(JB)]
            xTs = [pp.tile([128, I], BF16, tag="xTs", name="xTs") for _ in range(NPAIR)]

            E_lo = [None] * JB
            E_hi = [None] * JB

            def v_proj(jc, vT, wv_t):
                js = slice(jc * 128, (jc + 1) * 128)
                ps = ps_rot.tile([128, S], F32, tag="rot", name="rot")
                for kc in range(KC):
                    for nh in range(2):
                        nsl = slice(nh * 512, (nh + 1) * 512)
                        nc.tensor.matmul(
                            ps[:, nsl], vT[kc][:, js], wv_t[kc][:, nsl],
                            start=(kc == 0), stop=(kc == KC - 1),
                        )
                nc.vector.tensor_copy(Vs[jc][:], ps[:])

            def scores_exp(g, jb, e_dst):
                """score matmuls for head pair g, j-block jb + exp."""
                jsl = slice(jb * 128, (jb + 1) * 128)
                sc = ps_rot.tile([128, S], F32, tag="rot", name="rot")
                nc.tensor.matmul(
                    sc[:, 0:512], KTs[g][0:64, jsl], QTs[g][0:64, :],
                    start=True, stop=True, tile_position=(0, 0),
                )
                nc.tensor.matmul(
                    sc[:, 512:1024], KTs[g][64:128, jsl], QTs[g][64:128, :],
                    start=True, stop=True, tile_position=(64, 0),
                )
                nc.scalar.activation(e_dst, sc[:], EXP, scale=SCALE)

            def alloc_e(jb):
                E_lo[jb] = pe_lo.tile([128, 4096], BF16, tag="Elo", name="Elo")
                E_hi[jb] = pe_hi.tile([128, 4096], BF16, tag="Ehi", name="Ehi")

            def a_phase1(vT, wv_t, q_proj, k_proj):
                """g-major over jb 0-3, with Q/K projections interleaved."""
                for jb in range(4):
                    alloc_e(jb)
                for g in range(NPAIR):
                    q_proj(g)
                    k_proj(g)
                    for jb in range(4):
                        dst = (E_lo[jb] if g < 4 else E_hi[jb])
                        off = (g % 4) * 1024
                        scores_exp(g, jb, dst[:, off : off + 1024])
                    if g % 2 == 1:
                        v_proj(g // 2, vT, wv_t)

            def a2_iter(jb, vT, wv_t):
                """jb-major (projections all done): one j-block's scores."""
                alloc_e(jb)
                for g in range(NPAIR):
                    dst = (E_lo[jb] if g < 4 else E_hi[jb])
                    off = (g % 4) * 1024
                    scores_exp(g, jb, dst[:, off : off + 1024])
                v_proj(jb, vT, wv_t)

            def b_phase(jb, xt_ps, first, last):
                # Z = sum of 16 exp planes, via big strided slice adds
                t2 = pscr.tile([128, 2048], BF16, tag="t2", name="t2")
                nc.vector.tensor_add(t2[:], E_lo[jb][:, 0:2048], E_lo[jb][:, 2048:4096])
                nc.vector.tensor_add(t2[:], t2[:], E_hi[jb][:, 0:2048])
                nc.vector.tensor_add(t2[:], t2[:], E_hi[jb][:, 2048:4096])
                nc.vector.tensor_add(t2[:, 0:1024], t2[:, 0:1024], t2[:, 1024:2048])
                zf = pz.tile([128, 512], F32, tag="zf", name="zf")
                nc.vector.tensor_add(zf[:], t2[:, 0:512], t2[:, 512:1024])
                rf = pz.tile([128, 512], F32, tag="rf", name="rf")
                nc.vector.reciprocal_approx_fast(rf[:], zf[:])
                rb = pz.tile([128, 512], BF16, tag="rb", name="rb", bufs=2)
                nc.vector.tensor_copy(rb[:], rf[:])
                rv8 = rb[:].unsqueeze(1).broadcast_to([128, 8, 512])
                nc.vector.tensor_mul(
                    E_lo[jb][:].rearrange("p (a b) -> p a b", a=8),
                    E_lo[jb][:].rearrange("p (a b) -> p a b", a=8),
                    rv8,
                )
                nc.vector.tensor_mul(
                    E_hi[jb][:].rearrange("p (a b) -> p a b", a=8),
                    E_hi[jb][:].rearrange("p (a b) -> p a b", a=8),
                    rv8,
                )
                if jb < 4:
                    # spill normalized pairs 4-7 attn; read back in the tail
                    nc.sync.dma_start(eh_d[jb * 128 : (jb + 1) * 128, :], E_hi[jb][:])
                for g in range(4):
                    for p in range(2):
                        h = 2 * g + p
                        nc.tensor.matmul(
                            xt_ps[g][p * 64 : (p + 1) * 64, :],
                            Vs[jb][:, h * DH : (h + 1) * DH],
                            E_lo[jb][:, (2 * g + p) * 512 : (2 * g + p + 1) * 512],
                            start=first, stop=last,
                            tile_position=(0, p * 64),
                        )

            with (
                tc.tile_pool(name="st_v", bufs=KC) as pv,
                tc.tile_pool(name="st_wv", bufs=KC) as pwv,
            ):
                vT = [pv.tile([128, S], BF16, tag="vT", name="vT") for _ in range(KC)]
                wv_t = [pwv.tile([128, D], BF16, tag="wv", name="wv") for _ in range(KC)]

                with (
                    tc.tile_pool(name="st_q", bufs=KC) as pq,
                    tc.tile_pool(name="st_k", bufs=KC) as pk,
                    tc.tile_pool(name="st_wq", bufs=3) as pwq,
                    tc.tile_pool(name="st_wk", bufs=3) as pwk,
                ):
                    qT = [pq.tile([128, I], BF16, tag="qT", name="qT") for _ in range(KC)]
                    kT = [pk.tile([128, S], BF16, tag="kT", name="kT") for _ in range(KC)]
                    # wq_g[g] holds head-pair g's weight chunk for every kc
                    # side by side: [128 din-in-chunk, kc*128 + dout]
                    wq_g = [pwq.tile([128, D], BF16, tag="wq", name="wq")
                            for _ in range(NPAIR)]
                    wk_g = [pwk.tile([128, D], BF16, tag="wk", name="wk")
                            for _ in range(NPAIR)]

                    def load_w_g(tiles, dram, g):
                        # one DMA: packed rows [(c p) x] -> tile [p (c x)],
                        # partition axis leading on both sides
                        nc.sync.dma_start(
                            tiles[g][:].rearrange("p (c x) -> p c x", c=KC),
                            dram[g * D : (g + 1) * D, :].rearrange(
                                "(c p) x -> p c x", c=KC
                            ),
                        )

                    # loads in consumption order; vT/wv ahead of the late
                    # wq/wk chunks so a stalled weight DMA can't starve V
                    load_w_g(wq_g, wqp_d, 0)
                    for c in range(KC):
                        cs = slice(c * 128, (c + 1) * 128)
                        nc.sync.dma_start(qT[c][:], q_d[cs, :])
                    load_w_g(wk_g, wkp_d, 0)
                    for c in range(KC):
                        cs = slice(c * 128, (c + 1) * 128)
                        nc.sync.dma_start(kT[c][:], k_d[cs, :])
                    load_w_g(wq_g, wqp_d, 1)
                    load_w_g(wk_g, wkp_d, 1)
                    for c in range(KC):
                        cs = slice(c * 128, (c + 1) * 128)
                        nc.sync.dma_start(vT[c][:], v_d[cs, :])
                        nc.sync.dma_start(wv_t[c][:], wv_d[cs, :])
                    for g in range(2, NPAIR):
                        load_w_g(wq_g, wqp_d, g)
                        load_w_g(wk_g, wkp_d, g)

                    def q_proj(g):
                        ps = ps_rot.tile([128, S], F32, tag="rot", name="rot")
                        for kc in range(KC):
                            nc.tensor.matmul(
                                ps[:, 0:I], wq_g[g][:, kc * 128 : (kc + 1) * 128], qT[kc][:],
                                start=(kc == 0), stop=(kc == KC - 1),
                            )
                        nc.vector.tensor_copy(QTs[g][:], ps[:, 0:I])

                    def k_proj(g):
                        ps = ps_rot.tile([128, S], F32, tag="rot", name="rot")
                        for kc in range(KC):
                            for nh in range(2):
                                nsl = slice(nh * 512, (nh + 1) * 512)
                                nc.tensor.matmul(
                                    ps[:, nsl], wk_g[g][:, kc * 128 : (kc + 1) * 128], kT[kc][:, nsl],
                                    start=(kc == 0), stop=(kc == KC - 1),
                                )
                        nc.scalar.copy(KTs[g][:], ps[:])

                    a_phase1(vT, wv_t, q_proj, k_proj)
                # st_q/st_k/st_wq/st_wk freed here

                xt_ps = [ps_xt.tile([128, I], F32, tag="xt", name="xt") for _ in range(4)]
                # interleave A2 (scores/exp jb 4-7 on PE/ACT) with B(0..3)
                # (DVE softmax chains + AV pairs 0-3); a2_iter(jb) issued
                # ahead of b_phase(jb-4) so the in-order PE queue never
                # head-of-line blocks on a DVE chain
                for jb in range(4, JB):
                    a2_iter(jb, vT, wv_t)
                    b_phase(jb - 4, xt_ps, first=(jb == 4), last=False)
            # st_v/st_wv freed here

            for jb in range(4, JB):
                b_phase(jb, xt_ps, first=False, last=(jb == JB - 1))
            for g in range(4):
                nc.scalar.copy(xTs[g][:], xt_ps[g][:])

            # ---- tail: AV pairs 4-7; jb 4-7 from SBUF, jb 0-3 from the
            # HBM spill (read back into a small rotation pool) ----------
            with tc.tile_pool(name="peb", bufs=4) as peb:
                xt_ps2 = [ps_xt.tile([128, I], F32, tag="xt", name="xt") for _ in range(4)]

                def av_hi(jb, src, first, last):
                    for g in range(4, NPAIR):
                        for p in range(2):
                            h = 2 * g + p
                            off = (2 * (g - 4) + p) * 512
                            nc.tensor.matmul(
                                xt_ps2[g - 4][p * 64 : (p + 1) * 64, :],
                                Vs[jb][:, h * DH : (h + 1) * DH],
                                src[:, off : off + 512],
                                start=first, stop=last,
                                tile_position=(0, p * 64),
                            )

                backs = []
                for jb in range(4):
                    eb = peb.tile([128, 4096], BF16, tag="eb", name="eb")
                    nc.sync.dma_start(eb[:], eh_d[jb * 128 : (jb + 1) * 128, :])
                    backs.append(eb)
                for jb in range(4, JB):
                    av_hi(jb, E_hi[jb][:], first=(jb == 4), last=False)
                for jb in range(4):
                    av_hi(jb, backs[jb][:], first=False, last=(jb == 3))
                for g in range(4, NPAIR):
                    nc.scalar.copy(xTs[g][:], xt_ps2[g - 4][:])

            # ---- output projection (wo loaded late) -------------------
            with (
                tc.tile_pool(name="pwo", bufs=KC) as pwo,
                tc.tile_pool(name="pout", bufs=2) as pout,
            ):
                wo_t = [pwo.tile([128, D], BF16, tag="wo", name="wo") for _ in range(KC)]
                for g in range(KC):
                    nc.sync.dma_start(wo_t[g][:], wo_d[g * 128 : (g + 1) * 128, :])
                for ic in range(I // 128):
                    isl = slice(ic * 128, (ic + 1) * 128)
                    ps = po_rot.tile([128, D], F32, tag="orot", name="orot")
                    for g in range(KC):
                        for nh in range(2):
                            nsl = slice(nh * 512, (nh + 1) * 512)
                            nc.tensor.matmul(
                                ps[:, nsl], xTs[g][:, isl], wo_t[g][:, nsl],
                                start=(g == 0), stop=(g == KC - 1),
                            )
                    of = pout.tile([128, D], F32, tag="outf", name="outf")
                    nc.scalar.copy(of[:], ps[:])
                    nc.sync.dma_start(out_d[isl, :], of[:])
    nc.compile()
    return nc


_NC_CACHE = {}


def _get_nc():
    if "nc" not in _NC_CACHE:
        _NC_CACHE["nc"] = _build()
    return _NC_CACHE["nc"]


def _reference_numpy(query, key, value, mask, Wq, bq, Wk, bk, Wv, bv, Wo, bo):
    """Fallback for masked / biased inputs (reference semantics)."""
    q = (query.reshape(B * S, D) @ Wq + bq).reshape(B, S, H, DH)
    k = (key.reshape(B * S, D) @ Wk + bk).reshape(B, S, H, DH)
    v = (value.reshape(B * S, D) @ Wv + bv).reshape(B, S, H, DH)
    scores = np.einsum("bihd,bjhd->bijh", q, k).astype(np.float32) * SCALE
    scores = np.where(mask[..., None] == 0, -np.inf, scores)
    m = scores.max(axis=-1, keepdims=True)
    e = np.exp(scores - m)
    attn = e / e.sum(axis=-1, keepdims=True)
    x = np.einsum("bijh,bjhd->bihd", attn, v).reshape(B, S, D)
    return (x.reshape(B * S, D) @ Wo + bo).reshape(B, S, D).astype(np.float32)


def _row_pack(t):
    """[KC*128, W] -> [128, KC*W]: per-partition contiguous rows."""
    bf = ml_dtypes.bfloat16
    w = t.shape[1]
    out = t.reshape(KC, 128, w).transpose(1, 0, 2).reshape(128, KC * w)
    return np.ascontiguousarray(out.astype(bf))


def _pack_wqk(w):
    """[D, D] -> [128, (g c x)] so head-pair g's block is one row-slice."""
    bf = ml_dtypes.bfloat16
    t = w.reshape(KC, 128, NPAIR, 128).transpose(1, 2, 0, 3).reshape(128, NPAIR * D)
    return np.ascontiguousarray(t.astype(bf))


def kernel(query, key, value, mask, Wq, bq, Wk, bk, Wv, bv, Wo, bo):
    query = np.asarray(query, np.float32)
    key = np.asarray(key, np.float32)
    value = np.asarray(value, np.float32)
    Wq, Wk, Wv, Wo = (np.asarray(w, np.float32) for w in (Wq, Wk, Wv, Wo))
    bq, bk, bv, bo = (np.asarray(b, np.float32) for b in (bq, bk, bv, bo))
    mask_np = np.asarray(mask)

    if (not np.all(mask_np != 0)) or bq.any() or bk.any() or bv.any() or bo.any():
        return _reference_numpy(
            query, key, value, mask_np, Wq, bq, Wk, bk, Wv, bv, Wo, bo
        )

    nc = _get_nc()

    kwb = [_row_pack(key[b].T) for b in range(B)]
    vwb = [_row_pack(value[b].T) for b in range(B)]
    wqw, wkw = _pack_wqk(Wq), _pack_wqk(Wk)
    wvw, wow = _row_pack(Wv), _row_pack(Wo)

    in_maps = []
    for c in range(NCORES):
        b, ih = divmod(c, 2)
        in_maps.append({
            "qw": _row_pack(query[b, ih * I : (ih + 1) * I, :].T),
            "kw": kwb[b], "vw": vwb[b],
            "wqw": wqw, "wkw": wkw, "wvw": wvw, "wow": wow,
        })

    res = run_bass_kernel_spmd(nc, in_maps, core_ids=list(range(NCORES)))
    global LAST_RESULT
    LAST_RESULT = res
    out = np.empty((B, S, D), np.float32)
    for c in range(NCORES):
        b, ih = divmod(c, 2)
        out[b, ih * I : (ih + 1) * I, :] = res.results[c]["out"]
    return out
